# revision 18
# baseline (speedup 1.0000x reference)
"""CLCE loss kernel for Trainium2 (8 NeuronCores, SPMD) — symmetric version.

Loss = 0.5 * cl + 0.5 * ce where
  cl_i = log(exp(slot0_i) + (T_i - P_i) + Z_i) - slot0_i
  T_i  = sum_j exp((xn_i . xn_j + 1) * 0.25)     <- O(N^2 D), on device
  P_i, slot0_i: same-class corrections, on host (exact, tiny)
  ce: cross-entropy of y_pred, on host in f64 (O(N*C), tiny)

Device: the N x N exp-sim row-sum exploits symmetry — only the upper
triangle of the 32x32 grid of 128-cells is computed (528 of 1024 cells).
Each computed strip contributes its row-sums via the Scalar engine's
accum_out, and its mirrored contribution via column-sums: exp tiles are
accumulated per column-tile on the Vector engine (bf16) and reduced
across partitions with one ones-matmul per column slot.

Uniform SPMD structure: every core runs the identical 9-item schedule
(diag staircase upper/lower + 7 off-diagonal 256-row half-blocks) over
6 data slots (4 column tiles + 2 flexible weight-row slots).  The host
chooses per-core slot contents so the 8 cores tile the triangle exactly:
  slots(c)  = [c, c+1, c+2, T3[c]]  (mod 8), T3 = [4,5,6,7,7,4,5,6]
  W halves  = difference-class patches (d2-h1, d3-h0, d3-h1, d4 halves)
Embeddings are fp8 (pre-scaled by S8) with DoubleRow matmuls, identical
quantization to what the host correction terms replicate.
"""

import os
from contextlib import ExitStack

import numpy as np

import concourse.bass as bass
import concourse.tile as tile
from concourse import bacc, mybir
from concourse.bass_utils import run_bass_kernel_spmd

N, D, C = 4096, 1024, 512
TAU = 0.5
LAMBD = 0.5
NCORES = 8
P = 128                    # partitions
KT = D // 256              # 4 DoubleRow contraction super-tiles
TW = 512                   # tile width (columns per tile slot)
S8 = 16.0                  # fp8 pre-scale for the embeddings
NSLOT = 6                  # 4 column tiles + 2 weight-row slots
NWARM = 5                  # PE warm-up matmuls

_F32 = mybir.dt.float32
_BF16 = mybir.dt.bfloat16
_FP8 = mybir.dt.float8e4
_EXP = mybir.ActivationFunctionType.Exp
_DR = mybir.MatmulPerfMode.DoubleRow

# ---------------- cover tables (validated exact) ----------------
T3 = [4, 5, 6, 7, 7, 4, 5, 6]


def _slots_of(c):
    return [c, (c + 1) % 8, (c + 2) % 8, T3[c]]


def _whalves_of(c):
    # (tile, half) for W halves 0..3 (slot4 m01, slot4 m23, slot5 m01, slot5 m23)
    return [((c - 2) % 8, 1), ((c - 2) % 8, 0), ((c - 1) % 8, 1),
            (T3[c] - 4, 0 if c < 4 else 1)]


# item: (row_slot, mlo, col_slot, kind); row_slot 0..3 = tile, 4..5 = W slots
ITEMS = [
    (0, 0, 0, 'DU'),   # I0 diag staircase rows m0,m1
    (0, 2, 0, 'DL'),   # I1 diag staircase rows m2,m3
    (0, 0, 1, 'OFF'),  # I2 (c -> c+1, h0)
    (1, 2, 2, 'OFF'),  # I3 (c+1 -> c+2, h1)
    (0, 0, 2, 'OFF'),  # I4 (c -> c+2, h0)
    (4, 0, 0, 'OFF'),  # I5 W patch, cols slot0
    (4, 2, 1, 'OFF'),  # I6 W patch, cols slot1
    (5, 0, 2, 'OFF'),  # I7 W patch, cols slot2
    (5, 2, 3, 'OFF'),  # I8 W patch, cols slot3
]
# device schedule order (I1 last: its final chunk has no colsum -> short tail)
ORDER = [0, 2, 5, 6, 3, 4, 7, 8, 1]


def _item_chunks(kind, mloc):
    """(mm_lo, mm_hi, cs_lo, cs_hi) column ranges for one m-group.
    [mm_lo, mm_hi) is computed+row-summed; [cs_lo, cs_hi) feeds colsum
    (strict-upper cells; diag 128-cells excluded).  cs_lo==cs_hi: none."""
    if kind == 'OFF':
        return (0, 512, 0, 512)
    if kind == 'DU':
        if mloc == 0:
            return (0, 512, 128, 512)
        return (128, 512, 256, 512)
    # DL
    if mloc == 0:
        return (256, 512, 384, 512)
    return (384, 512, 512, 512)


def _item_rows(c, item, mloc):
    """Global start row of the 128-row group (item, mloc) computes."""
    slot, mlo = item[0], item[1]
    if slot < 4:
        return _slots_of(c)[slot] * 512 + (mlo + mloc) * 128
    t, h = _whalves_of(c)[(slot - 4) * 2 + mlo // 2]
    return t * 512 + h * 256 + mloc * 128


def _emission_chunks():
    """(item_idx, mloc) in device emission order = accum col order."""
    return [(it, mloc) for it in ORDER for mloc in (0, 1)]


NRS = len(_emission_chunks())   # rowsum output columns (18)


# ---------------- device kernel ----------------
def _build_kernel(tc, xt, out1, out2):
    nc = tc.nc
    act_scale = 0.5 * TAU / (S8 * S8)
    with ExitStack() as ctx:
        pers = ctx.enter_context(tc.tile_pool(name="pers", bufs=1))
        epool = ctx.enter_context(tc.tile_pool(name="epool", bufs=3))
        psum = ctx.enter_context(
            tc.tile_pool(name="psum", bufs=6, space=bass.MemorySpace.PSUM)
        )

        SLOT = [
            pers.tile([P, KT, 2, TW], _FP8, name=f"slot{s}", tag=f"slot{s}")
            for s in range(NSLOT)
        ]
        OUTSB = pers.tile([P, NRS], _F32)
        CSSB = pers.tile([1, NRS * TW], _F32)
        bias_s = pers.tile([P, 1], _F32)
        bias_z = pers.tile([P, 1], _F32)
        warm = pers.tile([P, 1], _F32)
        ZW = pers.tile([P, 512], _BF16)

        nc.gpsimd.memset(ZW[:], 0.0)
        nc.gpsimd.memset(bias_s[:], 0.5 * TAU)
        nc.gpsimd.memset(bias_z[:], 0.0)
        # warm the exp table before any data lands
        nc.scalar.activation(warm[:], bias_z[:], _EXP, bias=bias_z[:], scale=1.0)

        # PE warm-up: dummy matmuls spanning the input-DMA latency flip the
        # HAM clock gate to 8/8 so the real stream runs at 2.4GHz
        wps = psum.tile([P, 512], _F32, tag="ps")
        for _ in range(NWARM):
            nc.tensor.matmul(wps[:, 0:512], ZW[:, 0:P], ZW[:], start=True,
                             stop=True)

        # input DMAs: single HWDGE queue in exact first-use order so the
        # critical first bytes never share SDMA round-robin slots.
        xt6 = xt.rearrange("p (s k i n) -> p s k i n", s=NSLOT, k=KT, i=2)
        for s in (0, 1, 4, 2, 5, 3):
            nc.sync.dma_start(SLOT[s][:], xt6[:, s])

        # main schedule: matmuls on PE, exp+rowsum on Scalar, colsum
        # (partition reduction of each exp tile) on the otherwise-idle GpSimd
        rs_col = 0
        for it in ORDER:
            row_slot, mlo, col_slot, kind = ITEMS[it]
            for mloc in (0, 1):
                mm_lo, mm_hi, cs_lo, cs_hi = _item_chunks(kind, mloc)
                mcol = mlo + mloc
                ps = psum.tile([P, 512], _F32, tag="ps")
                for k in range(KT):
                    nc.tensor.matmul(
                        ps[:, mm_lo:mm_hi],
                        SLOT[row_slot][:, k, :, mcol * P:(mcol + 1) * P],
                        SLOT[col_slot][:, k, :, mm_lo:mm_hi],
                        start=(k == 0),
                        stop=(k == KT - 1),
                        perf_mode=_DR,
                    )
                et = epool.tile([P, 512], _BF16, tag="et")
                nc.scalar.activation(
                    et[:, 0:mm_hi - mm_lo], ps[:, mm_lo:mm_hi], _EXP,
                    bias=bias_s[:], scale=act_scale,
                    accum_out=OUTSB[:, rs_col:rs_col + 1],
                )
                if cs_lo < cs_hi:
                    w = cs_hi - cs_lo
                    nc.gpsimd.tensor_reduce(
                        CSSB[0:1, rs_col * TW:rs_col * TW + w],
                        et[:, cs_lo - mm_lo:cs_hi - mm_lo],
                        axis=mybir.AxisListType.C,
                        op=mybir.AluOpType.add,
                    )
                rs_col += 1

        nc.scalar.dma_start(out1[:], OUTSB[:])
        nc.sync.dma_start(out2[:], CSSB[:])


_NC_CACHE = None


def _get_nc():
    global _NC_CACHE
    if _NC_CACHE is None:
        nc = bacc.Bacc(
            "TRN2", target_bir_lowering=False, debug=False,
            enable_asserts=False, num_devices=NCORES,
        )
        xt_d = nc.dram_tensor("xt", [P, NSLOT * KT * 2 * TW], _FP8,
                              kind="ExternalInput")
        out1_d = nc.dram_tensor("out1", [P, NRS], _F32, kind="ExternalOutput")
        out2_d = nc.dram_tensor("out2", [1, NRS * TW], _F32,
                                kind="ExternalOutput")
        with tile.TileContext(nc) as tc:
            _build_kernel(tc, xt_d.ap(), out1_d.ap(), out2_d.ap())
        nc.compile()
        _NC_CACHE = nc
    return _NC_CACHE


def _pack_cols(cols):
    """[D, 512] fp8 column block -> [P, 4096] with DoubleRow pairing:
    partition p, byte (k*2 + i)*512 + n  <->  contraction index
    k*256 + 128*i + p  of column n."""
    q = cols.reshape(KT, 2, P, TW).transpose(2, 0, 1, 3).reshape(P, KT * 2 * TW)
    return np.ascontiguousarray(q)


def _run_device(zq8, trace=False):
    """zq8: [D, N] fp8 pre-scaled quantized embeddings (as fp8 np dtype)."""
    in_maps = []
    for c in range(NCORES):
        slots = _slots_of(c)
        wh = _whalves_of(c)
        parts = []
        for s in range(4):
            t = slots[s]
            parts.append(_pack_cols(zq8[:, t * 512:(t + 1) * 512]))
        for w0 in (0, 2):  # slots 4, 5
            blk = np.concatenate(
                [zq8[:, wh[w0 + j][0] * 512 + wh[w0 + j][1] * 256:][:, :256]
                 for j in range(2)], axis=1)
            parts.append(_pack_cols(np.ascontiguousarray(blk)))
        in_maps.append({"xt": np.concatenate(parts, axis=1)})
    res = run_bass_kernel_spmd(
        _get_nc(), in_maps, core_ids=list(range(NCORES)), trace=trace,
    )
    T = np.zeros(N, np.float64)
    chunks = _emission_chunks()
    for c, r in enumerate(res.results):
        o1 = r["out1"].astype(np.float64)   # [P, NRS]
        o2 = r["out2"].astype(np.float64)   # [1, NRS*TW]
        slots = _slots_of(c)
        for col, (it, mloc) in enumerate(chunks):
            item = ITEMS[it]
            r0 = _item_rows(c, item, mloc)
            T[r0:r0 + P] += o1[:, col]
            mm_lo, mm_hi, cs_lo, cs_hi = _item_chunks(item[3], mloc)
            if cs_lo < cs_hi:
                ct = slots[item[2]]
                T[ct * 512 + cs_lo:ct * 512 + cs_hi] += \
                    o2[0, col * TW:col * TW + (cs_hi - cs_lo)]
    return T, res


def kernel(layer_embeds, y_true, y_pred):
    x = np.asarray(layer_embeds, dtype=np.float32)
    yt = np.asarray(y_true).astype(np.int64)
    yp = np.asarray(y_pred, dtype=np.float32)

    # normalize rows (torch-style eps clip)
    norms = np.maximum(
        np.sqrt((x.astype(np.float64) ** 2).sum(1, keepdims=True)), 1e-8
    )
    xn = (x / norms).astype(np.float32)
    fp8np = mybir.dt.np(_FP8)
    zq8 = np.ascontiguousarray((xn.T * S8).astype(np.float32)).astype(fp8np)

    trace = bool(int(os.environ.get("CLCE_TRACE", "0")))
    T, res = _run_device(zq8, trace=trace)
    if trace:
        kernel.last_results = res

    # --- host-side small terms ---
    # P_ must match what the device summed for the same-class entries, i.e.
    # the fp8-quantized sim values, so quantize the same way here.
    xq = zq8.astype(np.float64).T / S8   # [N, D] device-visible xn
    counts = np.bincount(yt, minlength=C)
    P_ = np.zeros(N, np.float64)
    slot0 = np.zeros(N, np.float64)
    for cval in np.unique(yt):
        idx = np.where(yt == cval)[0]
        subq = xq[idx]
        sq = (subq @ subq.T + 1.0) * (0.5 * TAU)
        P_[idx] = np.exp(sq).sum(1)
        if len(idx) >= 2:
            # slot0 feeds the final formula directly -> full precision
            sub = xn[idx].astype(np.float64)
            s = (sub @ sub.T + 1.0) * (0.5 * TAU)
            firstpos = np.where(np.arange(len(idx)) == 0, 1, 0)
            slot0[idx] = s[np.arange(len(idx)), firstpos]

    num_neg = N - counts[yt]
    S = T - P_
    Z = (2 * N - 2 - num_neg).astype(np.float64)
    cl = (np.log(np.exp(slot0) + S + Z) - slot0).mean()

    # cross-entropy in f64 on host (O(N*C))
    ypd = yp.astype(np.float64)
    mp = ypd.max(axis=1, keepdims=True)
    lse = np.log(np.exp(ypd - mp).sum(axis=1)) + mp[:, 0]
    ce = (lse - ypd[np.arange(N), yt]).mean()

    loss = LAMBD * cl + (1.0 - LAMBD) * ce
    return np.asarray(loss, dtype=np.float32)


# revision 24
# speedup vs baseline: 26.0880x; 26.0880x over previous
"""CLCE loss kernel for Trainium2 (8 NeuronCores, SPMD) — symmetric version.

Loss = 0.5 * cl + 0.5 * ce where
  cl_i = log(exp(slot0_i) + (T_i - P_i) + Z_i) - slot0_i
  T_i  = sum_j exp((xn_i . xn_j + 1) * 0.25)     <- O(N^2 D), on device
  P_i, slot0_i: same-class corrections, on host (exact, tiny)
  ce: cross-entropy of y_pred, on host in f64 (O(N*C), tiny)

Device: the N x N exp-sim row-sum exploits symmetry — only the upper
triangle of the 32x32 grid of 128-cells is computed (528 of 1024 cells).
Each computed strip contributes its row-sums via the Scalar engine's
accum_out, and its mirrored contribution via column-sums: exp tiles are
accumulated per column-tile on the Vector engine (bf16) and reduced
across partitions with one ones-matmul per column slot.

Uniform SPMD structure: every core runs the identical 9-item schedule
(diag staircase upper/lower + 7 off-diagonal 256-row half-blocks) over
6 data slots (4 column tiles + 2 flexible weight-row slots).  The host
chooses per-core slot contents so the 8 cores tile the triangle exactly:
  slots(c)  = [c, c+1, c+2, T3[c]]  (mod 8), T3 = [4,5,6,7,7,4,5,6]
  W halves  = difference-class patches (d2-h1, d3-h0, d3-h1, d4 halves)
Embeddings are fp8 (pre-scaled by S8) with DoubleRow matmuls, identical
quantization to what the host correction terms replicate.
"""

import os
from contextlib import ExitStack

import numpy as np

import concourse.bass as bass
import concourse.tile as tile
from concourse import bacc, mybir
from concourse.bass_utils import run_bass_kernel_spmd

N, D, C = 4096, 1024, 512
TAU = 0.5
LAMBD = 0.5
NCORES = 8
P = 128                    # partitions
KT = D // 256              # 4 DoubleRow contraction super-tiles
TW = 512                   # tile width (columns per tile slot)
S8 = 16.0                  # fp8 pre-scale for the embeddings
NSLOT = 6                  # 4 column tiles + 2 weight-row slots
NWARM = 5                  # PE warm-up matmuls

_F32 = mybir.dt.float32
_BF16 = mybir.dt.bfloat16
_FP8 = mybir.dt.float8e4
_EXP = mybir.ActivationFunctionType.Exp
_DR = mybir.MatmulPerfMode.DoubleRow

# ---------------- cover tables (validated exact) ----------------
T3 = [4, 5, 6, 7, 7, 4, 5, 6]


def _slots_of(c):
    return [c, (c + 1) % 8, (c + 2) % 8, T3[c]]


def _whalves_of(c):
    # (tile, half) for W halves 0..3 (slot4 m01, slot4 m23, slot5 m01, slot5 m23)
    return [((c - 2) % 8, 1), ((c - 2) % 8, 0), ((c - 1) % 8, 1),
            (T3[c] - 4, 0 if c < 4 else 1)]


# item: (row_slot, mlo, col_slot, kind); row_slot 0..3 = tile, 4..5 = W slots
ITEMS = [
    (0, 0, 0, 'DU'),   # I0 diag staircase rows m0,m1
    (0, 2, 0, 'DL'),   # I1 diag staircase rows m2,m3
    (0, 0, 1, 'OFF'),  # I2 (c -> c+1, h0)
    (1, 2, 2, 'OFF'),  # I3 (c+1 -> c+2, h1)
    (0, 0, 2, 'OFF'),  # I4 (c -> c+2, h0)
    (4, 0, 0, 'OFF'),  # I5 W patch, cols slot0
    (4, 2, 1, 'OFF'),  # I6 W patch, cols slot1
    (5, 0, 2, 'OFF'),  # I7 W patch, cols slot2
    (5, 2, 3, 'OFF'),  # I8 W patch, cols slot3
]
# device schedule order (I1 last: its final chunk has no colsum -> short tail)
ORDER = [0, 2, 5, 6, 3, 4, 7, 8, 1]


def _item_chunks(kind, mloc):
    """(mm_lo, mm_hi, cs_lo, cs_hi) column ranges for one m-group.
    [mm_lo, mm_hi) is computed+row-summed; [cs_lo, cs_hi) feeds colsum
    (strict-upper cells; diag 128-cells excluded).  cs_lo==cs_hi: none."""
    if kind == 'OFF':
        return (0, 512, 0, 512)
    if kind == 'DU':
        if mloc == 0:
            return (0, 512, 128, 512)
        return (128, 512, 256, 512)
    # DL
    if mloc == 0:
        return (256, 512, 384, 512)
    return (384, 512, 512, 512)


def _item_rows(c, item, mloc):
    """Global start row of the 128-row group (item, mloc) computes."""
    slot, mlo = item[0], item[1]
    if slot < 4:
        return _slots_of(c)[slot] * 512 + (mlo + mloc) * 128
    t, h = _whalves_of(c)[(slot - 4) * 2 + mlo // 2]
    return t * 512 + h * 256 + mloc * 128


def _emission_chunks():
    """(item_idx, mloc) in device emission order = accum col order."""
    return [(it, mloc) for it in ORDER for mloc in (0, 1)]


NRS = len(_emission_chunks())   # rowsum output columns (18)


# ---------------- device kernel ----------------
def _build_kernel(tc, xt, out1, out2):
    nc = tc.nc
    act_scale = 0.5 * TAU / (S8 * S8)
    with ExitStack() as ctx:
        pers = ctx.enter_context(tc.tile_pool(name="pers", bufs=1))
        epool = ctx.enter_context(tc.tile_pool(name="epool", bufs=3))
        psum = ctx.enter_context(
            tc.tile_pool(name="psum", bufs=3, space=bass.MemorySpace.PSUM)
        )
        cs0psum = ctx.enter_context(
            tc.tile_pool(name="cs0psum", bufs=1, space=bass.MemorySpace.PSUM)
        )
        cspsum = ctx.enter_context(
            tc.tile_pool(name="cspsum", bufs=1, space=bass.MemorySpace.PSUM)
        )

        SLOT = [
            pers.tile([P, KT, 2, TW], _FP8, name=f"slot{s}", tag=f"slot{s}")
            for s in range(NSLOT)
        ]
        A = [pers.tile([P, TW], _BF16, name=f"acc{s}") for s in range(4)]
        OUTSB = pers.tile([P, NRS], _F32)
        CSSB = pers.tile([1, 4 * TW], _F32)
        bias_s = pers.tile([P, 1], _F32)
        bias_z = pers.tile([P, 1], _F32)
        warm = pers.tile([P, 1], _F32)
        ones = pers.tile([P, 1], _BF16)
        ZW = pers.tile([P, 512], _BF16)

        nc.gpsimd.memset(ZW[:], 0.0)
        nc.gpsimd.memset(bias_s[:], 0.5 * TAU)
        nc.gpsimd.memset(bias_z[:], 0.0)
        nc.gpsimd.memset(ones[:], 1.0)
        for s in range(4):
            nc.gpsimd.memset(A[s][:], 0.0)
        # warm the exp table before any data lands
        nc.scalar.activation(warm[:], bias_z[:], _EXP, bias=bias_z[:], scale=1.0)

        # PE warm-up: dummy matmuls spanning the input-DMA latency flip the
        # HAM clock gate to 8/8 so the real stream runs at 2.4GHz
        wps = psum.tile([P, 2 * TW], _F32, tag="ps")
        for _ in range(NWARM):
            nc.tensor.matmul(wps[:, 0:512], ZW[:, 0:P], ZW[:], start=True,
                             stop=True)

        # input DMAs: single HWDGE queue in exact first-use order so the
        # critical first bytes never share SDMA round-robin slots.
        xt6 = xt.rearrange("p (s k i n) -> p s k i n", s=NSLOT, k=KT, i=2)
        for s in (0, 1, 4, 2, 5, 3):
            nc.sync.dma_start(SLOT[s][:], xt6[:, s])

        # main schedule.  Engine split: matmuls on PE; exp on Scalar (one
        # merged 1024-wide activation for OFF items); row-sums via Scalar
        # accum_out for diag items, via DVE tensor_reduce for OFF items;
        # col-sums via DVE accumulators + one ones-matmul per column slot
        # (I8's accumulator is bypassed straight into its psum).
        cs_tile = {}

        def cs_mm(target, rhs, start, stop):
            nc.tensor.matmul(target, ones[:, 0:1], rhs, start=start, stop=stop)

        rs_col = 0
        for it in ORDER:
            row_slot, mlo, col_slot, kind = ITEMS[it]
            ps = psum.tile([P, 2 * TW], _F32, tag="ps")
            et = epool.tile([P, 2 * TW], _BF16, tag="et")
            info = []
            for mloc in (0, 1):
                mm_lo, mm_hi, cs_lo, cs_hi = _item_chunks(kind, mloc)
                base = mloc * TW
                mcol = mlo + mloc
                for k in range(KT):
                    nc.tensor.matmul(
                        ps[:, base + mm_lo:base + mm_hi],
                        SLOT[row_slot][:, k, :, mcol * P:(mcol + 1) * P],
                        SLOT[col_slot][:, k, :, mm_lo:mm_hi],
                        start=(k == 0),
                        stop=(k == KT - 1),
                        perf_mode=_DR,
                    )
                info.append((base, mm_lo, mm_hi, cs_lo, cs_hi))
            if kind == 'OFF':
                # one merged exp, row-sums on DVE
                nc.scalar.activation(et[:, 0:2 * TW], ps[:, 0:2 * TW], _EXP,
                                     bias=bias_s[:], scale=act_scale)
                for (base, mm_lo, mm_hi, cs_lo, cs_hi) in info:
                    nc.vector.tensor_add(
                        A[col_slot][:, :], A[col_slot][:, :],
                        et[:, base:base + TW])
                for (base, mm_lo, mm_hi, cs_lo, cs_hi) in info:
                    nc.vector.tensor_reduce(
                        OUTSB[:, rs_col:rs_col + 1], et[:, base:base + TW],
                        axis=mybir.AxisListType.X, op=mybir.AluOpType.add)
                    rs_col += 1
            else:
                # diag staircase: per-m exp with Scalar accum row-sums
                for (base, mm_lo, mm_hi, cs_lo, cs_hi) in info:
                    nc.scalar.activation(
                        et[:, base:base + mm_hi - mm_lo],
                        ps[:, base + mm_lo:base + mm_hi], _EXP,
                        bias=bias_s[:], scale=act_scale,
                        accum_out=OUTSB[:, rs_col:rs_col + 1])
                    rs_col += 1
                    if cs_lo < cs_hi:
                        eo = base + cs_lo - mm_lo
                        w = cs_hi - cs_lo
                        if kind == 'DL':
                            cs_tile['dl_et'] = (et, eo, w)
                        else:
                            nc.vector.tensor_add(
                                A[0][:, cs_lo:cs_hi], A[0][:, cs_lo:cs_hi],
                                et[:, eo:eo + w])
            # colsum reductions, placed one item after each accumulator
            # closes so the PE never waits on the DVE adds
            if it == 6:     # after I6: A0 (DU + I5) closed at I5
                c0 = cs0psum.tile([P, TW], _F32, tag="cs0")
                cs_tile['cs0'] = c0
                cs_mm(c0[0:1, :], A[0][:, :], start=True, stop=False)
            elif it == 3:   # after I3: A1 (I2 + I6) closed at I6
                c1 = cspsum.tile([P, TW], _F32, tag="cs")
                cs_mm(c1[0:1, :], A[1][:, :], start=True, stop=True)
                nc.scalar.activation(CSSB[0:1, TW:2 * TW], c1[0:1, :],
                                     mybir.ActivationFunctionType.Copy)
            elif it == 8:   # after I8: A2 (I3 + I4 + I7) closed at I7
                c2 = cspsum.tile([P, TW], _F32, tag="cs")
                cs_mm(c2[0:1, :], A[2][:, :], start=True, stop=True)
                nc.scalar.activation(CSSB[0:1, 2 * TW:3 * TW], c2[0:1, :],
                                     mybir.ActivationFunctionType.Copy)
            elif it == 1:   # tail: CS3 (A3 = I8, closed during I1's matmuls)
                c3 = cspsum.tile([P, TW], _F32, tag="cs")
                cs_mm(c3[0:1, :], A[3][:, :], start=True, stop=True)
                nc.vector.tensor_copy(CSSB[0:1, 3 * TW:4 * TW], c3[0:1, :])
                det, eo, w = cs_tile['dl_et']
                cs_mm(cs_tile['cs0'][0:1, 384:512], det[:, eo:eo + w],
                      start=False, stop=True)
                nc.vector.tensor_copy(CSSB[0:1, 0:TW], cs_tile['cs0'][0:1, :])

        nc.scalar.dma_start(out1[:], OUTSB[:])
        nc.sync.dma_start(out2[:], CSSB[:])


_NC_CACHE = None


def _get_nc():
    global _NC_CACHE
    if _NC_CACHE is None:
        nc = bacc.Bacc(
            "TRN2", target_bir_lowering=False, debug=False,
            enable_asserts=False, num_devices=NCORES,
        )
        xt_d = nc.dram_tensor("xt", [P, NSLOT * KT * 2 * TW], _FP8,
                              kind="ExternalInput")
        out1_d = nc.dram_tensor("out1", [P, NRS], _F32, kind="ExternalOutput")
        out2_d = nc.dram_tensor("out2", [1, 4 * TW], _F32,
                                kind="ExternalOutput")
        with tile.TileContext(nc) as tc:
            _build_kernel(tc, xt_d.ap(), out1_d.ap(), out2_d.ap())
        nc.compile()
        _NC_CACHE = nc
    return _NC_CACHE


def _pack_cols(cols):
    """[D, 512] fp8 column block -> [P, 4096] with DoubleRow pairing:
    partition p, byte (k*2 + i)*512 + n  <->  contraction index
    k*256 + 128*i + p  of column n."""
    q = cols.reshape(KT, 2, P, TW).transpose(2, 0, 1, 3).reshape(P, KT * 2 * TW)
    return np.ascontiguousarray(q)


def _run_device(zq8, trace=False):
    """zq8: [D, N] fp8 pre-scaled quantized embeddings (as fp8 np dtype)."""
    in_maps = []
    for c in range(NCORES):
        slots = _slots_of(c)
        wh = _whalves_of(c)
        parts = []
        for s in range(4):
            t = slots[s]
            parts.append(_pack_cols(zq8[:, t * 512:(t + 1) * 512]))
        for w0 in (0, 2):  # slots 4, 5
            blk = np.concatenate(
                [zq8[:, wh[w0 + j][0] * 512 + wh[w0 + j][1] * 256:][:, :256]
                 for j in range(2)], axis=1)
            parts.append(_pack_cols(np.ascontiguousarray(blk)))
        in_maps.append({"xt": np.concatenate(parts, axis=1)})
    res = run_bass_kernel_spmd(
        _get_nc(), in_maps, core_ids=list(range(NCORES)), trace=trace,
    )
    T = np.zeros(N, np.float64)
    chunks = _emission_chunks()
    for c, r in enumerate(res.results):
        o1 = r["out1"].astype(np.float64)   # [P, NRS]
        o2 = r["out2"].astype(np.float64)   # [1, 4*TW]
        slots = _slots_of(c)
        for col, (it, mloc) in enumerate(chunks):
            r0 = _item_rows(c, ITEMS[it], mloc)
            T[r0:r0 + P] += o1[:, col]
        for s in range(4):
            t = slots[s]
            T[t * 512:(t + 1) * 512] += o2[0, s * TW:(s + 1) * TW]
    return T, res


def kernel(layer_embeds, y_true, y_pred):
    x = np.asarray(layer_embeds, dtype=np.float32)
    yt = np.asarray(y_true).astype(np.int64)
    yp = np.asarray(y_pred, dtype=np.float32)

    # normalize rows (torch-style eps clip)
    norms = np.maximum(
        np.sqrt((x.astype(np.float64) ** 2).sum(1, keepdims=True)), 1e-8
    )
    xn = (x / norms).astype(np.float32)
    fp8np = mybir.dt.np(_FP8)
    zq8 = np.ascontiguousarray((xn.T * S8).astype(np.float32)).astype(fp8np)

    trace = bool(int(os.environ.get("CLCE_TRACE", "0")))
    T, res = _run_device(zq8, trace=trace)
    if trace:
        kernel.last_results = res

    # --- host-side small terms ---
    # P_ must match what the device summed for the same-class entries, i.e.
    # the fp8-quantized sim values, so quantize the same way here.
    xq = zq8.astype(np.float64).T / S8   # [N, D] device-visible xn
    counts = np.bincount(yt, minlength=C)
    P_ = np.zeros(N, np.float64)
    slot0 = np.zeros(N, np.float64)
    for cval in np.unique(yt):
        idx = np.where(yt == cval)[0]
        subq = xq[idx]
        sq = (subq @ subq.T + 1.0) * (0.5 * TAU)
        P_[idx] = np.exp(sq).sum(1)
        if len(idx) >= 2:
            # slot0 feeds the final formula directly -> full precision
            sub = xn[idx].astype(np.float64)
            s = (sub @ sub.T + 1.0) * (0.5 * TAU)
            firstpos = np.where(np.arange(len(idx)) == 0, 1, 0)
            slot0[idx] = s[np.arange(len(idx)), firstpos]

    num_neg = N - counts[yt]
    S = T - P_
    Z = (2 * N - 2 - num_neg).astype(np.float64)
    cl = (np.log(np.exp(slot0) + S + Z) - slot0).mean()

    # cross-entropy in f64 on host (O(N*C))
    ypd = yp.astype(np.float64)
    mp = ypd.max(axis=1, keepdims=True)
    lse = np.log(np.exp(ypd - mp).sum(axis=1)) + mp[:, 0]
    ce = (lse - ypd[np.arange(N), yt]).mean()

    loss = LAMBD * cl + (1.0 - LAMBD) * ce
    return np.asarray(loss, dtype=np.float32)
